# revision 20
# baseline (speedup 1.0000x reference)
"""Differential attention kernel for Trainium2 (8 NeuronCores).

v3.1: reassociated algebra. out = diff_attn @ V @ Wo is computed as
p @ (V @ Wo): the [S,S] attention map multiplies the precomputed
[S, D_MODEL] matrix VW = V @ Wo instead of the [S, 8192] V — a 4x FLOP
reduction on the attention side.

Sharding: 2 batch groups x 4 cores. Within a group, core g computes
  - VW rows [g*512, (g+1)*512) = (x_rows @ Wv) @ Wo fully locally
    (contracting all 8192 v on-core; f32 PSUM accumulation, bf16 evict),
  - then a 4-rank bf16 AllGather assembles the full VW [2048, 512],
  - attention (scores/softmax/combine/transpose) for q rows
    [g*512, (g+1)*512) only, overlapped with the AllGather,
  - p @ VW for its q quarter -> out tile [512, 512].
Host concatenates the 8 disjoint output tiles; bv/bo fold into a
constant host-side correction using sum_k(diff_attn[q,:]) == 1 - lam.

Phase order puts the V-chain first so the AllGather window is covered
by the qkv projection + scores/softmax work.
"""

import math

import numpy as np
import ml_dtypes

import concourse.bass as bass
from concourse import bacc
import concourse.mybir as mybir
import concourse.tile as tile
from concourse import bass_utils
from concourse.bass import ts, ds
from concourse.masks import make_identity

# Problem shapes (hardcoded per harness contract).
B = 2
S = 2048
D = 512
VDIM = 8192
DM = 512
P = 128
G = 4                 # cores per batch group
SQ = S // G           # 512 q (and VW k) rows per core
SCALE = 1.0 / math.sqrt(64.0)
LAMBDA_INIT = 0.8
LAYER_INDEX = 0

F32 = mybir.dt.float32
F32R = mybir.dt.float32r
BF16 = mybir.dt.bfloat16
EXP = mybir.ActivationFunctionType.Exp
IDENT = mybir.ActivationFunctionType.Identity
AXX = mybir.AxisListType.X

KD = D // P           # 4 contraction chunks of the input dim
MQ = (2 * D) // P     # 8 m-chunks of qkv output dim
SN = S // 512         # 4 free chunks of S
NKC = S // P          # 16 k-chunks of 128
QB = SQ // P          # 4 q-blocks per core
VB = VDIM // 1024     # 8 v-blocks of 1024
VC8 = 1024 // P       # 8 v-chunks of 128 per block
KQ = SQ // P          # 4 k-chunks of the core's VW quarter


def kernel_body(tc, xT, xTq, xq_bf, wqkv, wv, wo, lamn, bq, out):
    nc = tc.nc
    if xT.dtype != F32R:
        xT = xT.bitcast(F32R)
    if xTq.dtype != F32R:
        xTq = xTq.bitcast(F32R)
    if wqkv.dtype != F32R:
        wqkv = wqkv.bitcast(F32R)
    with (
        tc.tile_pool(name="persist", bufs=1) as persist,
        tc.tile_pool(name="dram", bufs=1, space="DRAM") as dram,
    ):
        _kernel_inner(tc, nc, persist, dram, xT, xTq, xq_bf, wqkv, wv, wo,
                      lamn, bq, out)


def _kernel_inner(tc, nc, persist, dram, xT, xTq, xq_bf, wqkv, wv, wo,
                  lamn, bq, out):
    lam_sb = persist.tile([P, 1], F32)       # holds -lam
    bq_sb = persist.tile([P, MQ], F32)
    ident_f32 = persist.tile([P, P], F32)
    ident_bf = persist.tile([P, P], BF16)
    qkvT_q = persist.tile([P, 4, SQ], F32R)   # Q1,Q1,Q2,Q2 for own quarter
    qkvT_k = persist.tile([P, 4, S], F32R)    # K1,K1,K2,K2 full
    ptile = persist.tile([P, NKC, SQ], BF16)  # p^T, [k_in, kc, q]
    vw_sb = persist.tile([P, NKC, DM], BF16)  # VW, [k_in, kc, m]
    r1s = [persist.tile([P, 1], F32, name=f"r1_{q}") for q in range(QB)]

    # tiny loads go on the ACT HWDGE queue to keep the SP queue head free
    nc.scalar.dma_start(lam_sb, lamn)
    nc.scalar.dma_start(bq_sb, bq)
    make_identity(nc, ident_f32)
    nc.vector.tensor_copy(ident_bf, ident_f32)

    cc_in = dram.tile([SQ, DM], BF16)
    cc_out = dram.tile([S, DM], BF16)

    # ---------------- phase 1: V-chain -> VW quarter -> AllGather ----------
    # The qkv-phase SBUF pool (qkvp) is opened alongside the V-chain pools so
    # its DMA loads can stream during V-chain compute (no SBUF-reuse barrier);
    # only the PSUM pools swap between phases.
    with (
        tc.tile_pool(name="vwp", bufs=1) as vwp,
        tc.tile_pool(name="vstr", bufs=2) as vstr,
        tc.tile_pool(name="qkvp", bufs=1) as qkvp,
    ):
      with (
        tc.tile_pool(name="vps", bufs=3, space="PSUM") as vps,
        tc.tile_pool(name="wps", bufs=1, space="PSUM") as wps,
      ):
        xq_sb = vwp.tile([P, KD, SQ], BF16)
        nc.sync.dma_start(xq_sb, xq_bf.rearrange("(dc p) s -> p dc s", p=P))
        vw_ps = [wps.tile([P, DM], F32, name=f"vw_ps_{kc}") for kc in range(KQ)]

        def emit_vt(vb, vc8, wv_sb, n):
            vt_ps = vps.tile([P, SQ], F32, tag="vtps", name=f"vtps_{vb}_{vc8}")
            for dc in range(KD):
                nc.tensor.matmul(
                    vt_ps, wv_sb[:, dc, ts(vc8, P)], xq_sb[:, dc],
                    start=(dc == 0), stop=(dc == KD - 1))
            vt_sb = vstr.tile([P, SQ], BF16, tag="vt", name=f"vt_{vb}_{vc8}")
            # alternate evictions between ACT and DVE to balance engines
            if n % 2 == 0:
                nc.scalar.activation(vt_sb, vt_ps, IDENT)
            else:
                nc.vector.tensor_copy(vt_sb, vt_ps)
            return vt_sb

        def emit_vw(vb, vc8, vt_sb, wo_sb):
            first = vb == 0 and vc8 == 0
            last = vb == VB - 1 and vc8 == VC8 - 1
            for kc in range(KQ):
                nc.tensor.matmul(
                    vw_ps[kc], vt_sb[:, ts(kc, P)], wo_sb[:, vc8, :],
                    start=first, stop=last)

        pend = []  # (vb, vc8, vt_sb, wo_sb) awaiting vw consume
        n = 0
        for vb in range(VB):
            wv_sb = vstr.tile([P, KD, 1024], BF16, tag="wv", bufs=4,
                              name=f"wv_{vb}")
            if vb == 0:
                # split the first block so the first matmuls start sooner
                for half in range(2):
                    nc.sync.dma_start(
                        wv_sb[:, :, ts(half, 512)],
                        wv[:, ds(half * 512, 512)]
                        .rearrange("(dc p) v -> p dc v", p=P))
            else:
                nc.sync.dma_start(
                    wv_sb,
                    wv[:, ts(vb, 1024)].rearrange("(dc p) v -> p dc v", p=P))
            wo_sb = vstr.tile([P, VC8, DM], BF16, tag="wo", bufs=4,
                              name=f"wo_{vb}")
            nc.sync.dma_start(wo_sb, wo[:, ds(vb * VC8 * DM, VC8 * DM)])
            for vc8 in range(VC8):
                vt_sb = emit_vt(vb, vc8, wv_sb, n)
                n += 1
                pend.append((vb, vc8, vt_sb, wo_sb))
                if len(pend) > 1:
                    emit_vw(*pend.pop(0))
        emit_vw(*pend.pop(0))

        # evict VW quarter -> cc_in, AllGather within the 4-core group
        vw_stage = vwp.tile([P, KQ, DM], BF16)
        for kc in range(KQ):
            # DVE evict: ACT stays free for the qkv evictions that follow
            nc.vector.tensor_copy(vw_stage[:, kc], vw_ps[kc])
        # store goes out on the ACT HWDGE queue (its wait is satisfied by
        # ACT's own predecessors, so it never stalls another engine)
        nc.scalar.dma_start(cc_in.rearrange("(kc p) m -> p kc m", p=P),
                            vw_stage)
        nc.gpsimd.collective_compute(
            "AllGather", mybir.AluOpType.bypass,
            ins=[cc_in[:]], outs=[cc_out[:]],
            replica_groups=[[0, 1, 2, 3], [4, 5, 6, 7]],
        )

        # qkv-phase loads: emitted while the V-chain is still computing
        xTs = qkvp.tile([P, KD, S], F32R)
        xTq_s = qkvp.tile([P, KD, SQ], F32R)
        wq_sb = qkvp.tile([P, KD, 2 * D], F32R)
        nc.sync.dma_start(wq_sb, wqkv.rearrange("(dc p) m -> p dc m", p=P))
        nc.sync.dma_start(xTq_s, xTq.rearrange("(dc p) s -> p dc s", p=P))
        for dc in range(KD):
            nc.sync.dma_start(xTs[:, dc], xT[ds(dc * P, P), :])
        # AG-gated gather-in load: issued last on the SP queue (after every
        # input load) so its semaphore wait stalls nothing that matters
        nc.sync.dma_start(vw_sb, cc_out.rearrange("(kc p) m -> p kc m", p=P))

      # ------------- phase 2: qkv projection (in the AllGather shadow) -----
      # (vps/wps closed; qps reuses their PSUM banks)
      with tc.tile_pool(name="qps", bufs=4, space="PSUM") as qps:
        # Q chunks (m 0..3) over own quarter
        for m in range(4):
            pt = qps.tile([P, SQ], F32, tag="ps")
            for dc in range(KD):
                nc.tensor.matmul(pt, wq_sb[:, dc, ts(m, P)], xTq_s[:, dc],
                                 start=(dc == 0), stop=(dc == KD - 1))
            nc.scalar.activation(qkvT_q[:, m], pt, IDENT,
                                 bias=bq_sb[:, m : m + 1])
        # K chunks (m 4..7) over full S; evictions alternate ACT/DVE so
        # neither engine serializes the following scores phase
        for sn in range(SN):
            for m in range(4, MQ):
                pt = qps.tile([P, 512], F32, tag="ps")
                for dc in range(KD):
                    nc.tensor.matmul(
                        pt, wq_sb[:, dc, ts(m, P)], xTs[:, dc, ts(sn, 512)],
                        start=(dc == 0), stop=(dc == KD - 1))
                if (sn * 4 + m) % 2 == 0:
                    nc.scalar.activation(qkvT_k[:, m - 4, ts(sn, 512)], pt,
                                         IDENT, bias=bq_sb[:, m : m + 1])
                else:
                    nc.vector.tensor_scalar_add(
                        qkvT_k[:, m - 4, ts(sn, 512)], pt,
                        bq_sb[:, m : m + 1])

    # ------- phase 3: scores/softmax/combine/transpose, then p @ VW -------
    with (
        tc.tile_pool(name="e1p", bufs=2) as e1p,
        tc.tile_pool(name="e2p", bufs=2) as e2p,
        tc.tile_pool(name="pbp", bufs=2) as pbp,
        tc.tile_pool(name="smallp", bufs=3) as smallp,
        tc.tile_pool(name="sps", bufs=2, space="PSUM") as sps,
        tc.tile_pool(name="ofp", bufs=2) as ofp,
    ):
        pend2 = []

        def emit_scores(qb):
            ets = []
            sums = []
            for mi in range(2):
                pool = e1p if mi == 0 else e2p
                et = pool.tile([P, S], BF16, tag=f"e{mi}", name=f"e{mi}_{qb}")
                st = smallp.tile([P, 2], F32, tag=f"sum{mi}",
                                 name=f"sum{mi}_{qb}")
                for half in range(2):
                    pt = sps.tile([P, 2, 512], F32, tag="ps",
                                  name=f"ps_{qb}_{mi}_{half}")
                    for knj in range(2):
                        kn = half * 2 + knj
                        for dc in range(2):
                            nc.tensor.matmul(
                                pt[:, knj],
                                qkvT_q[:, 2 * mi + dc, ts(qb, P)],
                                qkvT_k[:, 2 * mi + dc, ts(kn, 512)],
                                start=(dc == 0), stop=(dc == 1))
                    nc.scalar.activation(
                        et[:, ts(half, 1024)],
                        pt.rearrange("p a b -> p (a b)"), EXP, scale=SCALE,
                        accum_out=st[:, half : half + 1])
                ets.append(et)
                sums.append(st)
            s1 = smallp.tile([P, 1], F32, tag="s1", name=f"s1_{qb}")
            nc.vector.reduce_sum(s1, sums[0], axis=AXX)
            nc.vector.reciprocal(r1s[qb], s1)
            s2 = smallp.tile([P, 1], F32, tag="s2", name=f"s2_{qb}")
            nc.vector.reduce_sum(s2, sums[1], axis=AXX)
            r2 = smallp.tile([P, 1], F32, tag="r2", name=f"r2_{qb}")
            nc.vector.reciprocal(r2, s2)
            u = smallp.tile([P, 1], F32, tag="u", name=f"u_{qb}")
            nc.vector.tensor_mul(u, s1, lam_sb)       # u = -lam*s1
            r2q = smallp.tile([P, 1], F32, tag="r2q", name=f"r2q_{qb}")
            nc.vector.tensor_mul(r2q, u, r2)          # r2q = -lam*s1/s2
            pend2.append((qb, ets, r2q))

        def emit_combine():
            qb, ets, r2q = pend2.pop(0)
            pb = pbp.tile([P, S], BF16, tag="pb", name=f"pb_{qb}")
            nc.vector.affine_then_add(pb, ets[1], ets[0], r2q, 0.0)
            for kc4 in range(NKC // 4):
                tp = sps.tile([P, 4, P], BF16, tag="tp", name=f"tp_{qb}_{kc4}")
                for j in range(4):
                    kc = kc4 * 4 + j
                    nc.tensor.matmul(tp[:, j], pb[:, ts(kc, P)], ident_bf,
                                     is_transpose=True)
                nc.vector.tensor_copy(ptile[:, ts(kc4, 4), ts(qb, P)], tp)

        for qb in range(QB):
            emit_scores(qb)
            if qb > 0:
                emit_combine()
        emit_combine()

        # ---------------- phase 4: p @ VW, final evict ----------------
        with tc.tile_pool(name="fps", bufs=2, space="PSUM") as fps:
            for qb in range(QB):
                ft = fps.tile([P, DM], F32, tag="f", name=f"f_{qb}")
                for kc in range(NKC):
                    nc.tensor.matmul(
                        ft, ptile[:, kc, ts(qb, P)], vw_sb[:, kc, :],
                        start=(kc == 0), stop=(kc == NKC - 1))
                ofsb = ofp.tile([P, DM], F32, tag="of", name=f"of_{qb}")
                nc.scalar.activation(ofsb, ft, IDENT, scale=r1s[qb])
                nc.scalar.dma_start(out[ds(qb * P, P), :], ofsb)


def build_module(n_iters=1):
    nc = bacc.Bacc("TRN2", target_bir_lowering=False, debug=False)
    xT = nc.dram_tensor("xT", (D, S), F32R, kind="ExternalInput").ap()
    xTq = nc.dram_tensor("xTq", (D, SQ), F32R, kind="ExternalInput").ap()
    xq_bf = nc.dram_tensor("xq_bf", (D, SQ), BF16, kind="ExternalInput").ap()
    wqkv = nc.dram_tensor("wqkv", (D, 2 * D), F32R, kind="ExternalInput").ap()
    wv = nc.dram_tensor("wv", (D, VDIM), BF16, kind="ExternalInput").ap()
    wo = nc.dram_tensor("wo", (P, (VDIM // P) * DM), BF16,
                        kind="ExternalInput").ap()
    lamn = nc.dram_tensor("lamn", (P, 1), F32, kind="ExternalInput").ap()
    bq = nc.dram_tensor("bq", (P, MQ), F32, kind="ExternalInput").ap()
    out = nc.dram_tensor("out", (SQ, DM), F32, kind="ExternalOutput").ap()
    with tile.TileContext(nc) as tc:
        for _ in range(n_iters):
            kernel_body(tc, xT, xTq, xq_bf, wqkv, wv, wo, lamn, bq, out)
    nc.compile()
    return nc


_NC = None


def _get_module():
    global _NC
    if _NC is None:
        _NC = build_module()
    return _NC


def host_prep(**inputs):
    """Host-side input prep: returns (in_maps, lam, host_bias)."""
    x = np.asarray(inputs["x"], np.float32)
    Wqkv = np.asarray(inputs["Wqkv"], np.float32)
    bqkv = np.asarray(inputs["bqkv"], np.float32)
    Wv = np.asarray(inputs["Wv"], np.float32)
    bv = np.asarray(inputs["bv"], np.float32)
    Wo = np.asarray(inputs["Wo"], np.float32)
    bo = np.asarray(inputs["bo"], np.float32)
    lq1 = np.asarray(inputs["lq1"], np.float32)
    lk1 = np.asarray(inputs["lk1"], np.float32)
    lq2 = np.asarray(inputs["lq2"], np.float32)
    lk2 = np.asarray(inputs["lk2"], np.float32)

    lam = float(
        np.exp(np.sum(lq1 * lk1, dtype=np.float32))
        - np.exp(np.sum(lq2 * lk2, dtype=np.float32))
        + (LAMBDA_INIT - 0.6 * math.exp(-0.3 * LAYER_INDEX))
    )
    bq_host = np.ascontiguousarray(bqkv.reshape(MQ, P).T)
    lam_host = np.full((P, 1), -lam, np.float32)

    wv_bf = Wv.astype(ml_dtypes.bfloat16)
    # Wo [8192, 512] -> [128, (vc m)] with vc-major per partition
    wo_bf = np.ascontiguousarray(
        Wo.reshape(VDIM // P, P, DM).transpose(1, 0, 2).reshape(P, -1)
    ).astype(ml_dtypes.bfloat16)

    in_maps = []
    for c in range(8):
        b, g = divmod(c, G)
        xTb = np.ascontiguousarray(x[b].T)
        xTq = np.ascontiguousarray(xTb[:, g * SQ : (g + 1) * SQ])
        in_maps.append({
            "xT": xTb,
            "xTq": xTq,
            "xq_bf": xTq.astype(ml_dtypes.bfloat16),
            "wqkv": np.ascontiguousarray(Wqkv),
            "wv": wv_bf,
            "wo": wo_bf,
            "lamn": lam_host,
            "bq": bq_host,
        })
    # sum_k diff_attn[q, :] == 1 - lam exactly, so bv and bo fold into a
    # constant per-output-column correction.
    host_bias = ((1.0 - lam) * bv) @ Wo + bo
    return in_maps, lam, host_bias.astype(np.float32)


def kernel(**inputs):
    in_maps, _lam, host_bias = host_prep(**inputs)
    nc = _get_module()
    res = None
    for attempt in range(3):
        try:
            res = bass_utils.run_bass_kernel_spmd(
                nc, in_maps, core_ids=list(range(8)))
            break
        except Exception:
            # transient NRT_EXEC_UNIT_UNRECOVERABLE flakes have been seen on
            # the first execution of a freshly compiled NEFF; retry
            if attempt == 2:
                raise
            import time
            time.sleep(2.0)
    out = np.empty((B, S, DM), np.float32)
    for c in range(8):
        b, g = divmod(c, G)
        out[b, g * SQ : (g + 1) * SQ, :] = res.results[c]["out"]
    out += host_bias
    return out


# revision 22
# speedup vs baseline: 3.1510x; 3.1510x over previous
"""Differential attention kernel for Trainium2 (8 NeuronCores).

v4: reassociated algebra + cross-body software pipelining.

out = diff_attn @ V @ Wo is computed as p @ (V @ Wo): the [S,S] attention
map multiplies the precomputed [S, D_MODEL] matrix VW = V @ Wo instead of
the [S, 8192] V — a 4x FLOP reduction on the attention side.

Sharding: 2 batch groups x 4 cores. Within a group, core g computes
  - VW rows [g*512, (g+1)*512) = (x_rows @ Wv) @ Wo fully locally
    (contracting all 8192 v on-core; f32 PSUM accumulation, bf16 evict),
  - a 4-rank bf16 AllGather assembles the full VW [2048, 512],
  - attention (scores/softmax/combine/transpose) for q rows
    [g*512, (g+1)*512) only, overlapped with the AllGather,
  - p @ VW for its q quarter -> out tile [512, 512].
Host concatenates the 8 disjoint output tiles; bv/bo fold into a constant
host-side correction using sum_k(diff_attn[q,:]) == 1 - lam.

kernel_body is a generator yielding twice, so the driver can emit body
i's AllGather-gated tail (p @ VW + final evict) *after* body i+1's
V-chain: the PE instruction queue is in-order, so a p@VW matmul waiting
on the AllGather semaphore would otherwise stall every following matmul.
With the tail deferred one body, the gather always completes long before
its consumers issue and the collective is fully hidden in steady state.
"""

import math

import numpy as np
import ml_dtypes

import concourse.bass as bass
from concourse import bacc
import concourse.mybir as mybir
import concourse.tile as tile
from concourse import bass_utils
from concourse.bass import ts, ds
from concourse.masks import make_identity

# Problem shapes (hardcoded per harness contract).
B = 2
S = 2048
D = 512
VDIM = 8192
DM = 512
P = 128
G = 4                 # cores per batch group
SQ = S // G           # 512 q (and VW k) rows per core
SCALE = 1.0 / math.sqrt(64.0)
LAMBDA_INIT = 0.8
LAYER_INDEX = 0

F32 = mybir.dt.float32
F32R = mybir.dt.float32r
BF16 = mybir.dt.bfloat16
EXP = mybir.ActivationFunctionType.Exp
IDENT = mybir.ActivationFunctionType.Identity
AXX = mybir.AxisListType.X

KD = D // P           # 4 contraction chunks of the input dim
MQ = (2 * D) // P     # 8 m-chunks of qkv output dim
SN = S // 512         # 4 free chunks of S
NKC = S // P          # 16 k-chunks of 128
QB = SQ // P          # 4 q-blocks per core
VB = VDIM // 1024     # 8 v-blocks of 1024
VC8 = 1024 // P       # 8 v-chunks of 128 per block
KQ = SQ // P          # 4 k-chunks of the core's VW quarter


def kernel_body(tc, it, xT, xq_bf, wqkv, wv, wo, lamn, bq, out):
    """Generator: yields after phase-1 emission (V-chain + AllGather) and
    after phase-3 (scores); the code after the second yield emits the
    AllGather-gated tail (p @ VW + final evict)."""
    nc = tc.nc
    sd = "left" if it % 2 == 0 else "right"
    with (
        tc.tile_pool(name=f"carry{it}", bufs=1, side=sd) as carry,
        tc.tile_pool(name=f"lite{it}", bufs=1, side=sd) as lite,
        tc.tile_pool(name=f"dram{it}", bufs=1, space="DRAM", side=sd) as dram,
    ):
        lam_sb = lite.tile([P, 1], F32)       # holds -lam
        bq_sb = lite.tile([P, MQ], F32)
        ident_f32 = lite.tile([P, P], F32)
        ident_bf = lite.tile([P, P], BF16)
        ptile = carry.tile([P, NKC, SQ], BF16)   # p^T, [k_in, kc, q]
        vw_sb = carry.tile([P, NKC, DM], BF16)   # VW, [k_in, kc, m]
        r1s = [carry.tile([P, 1], F32, name=f"r1_{q}") for q in range(QB)]

        nc.scalar.dma_start(lam_sb, lamn)
        nc.scalar.dma_start(bq_sb, bq)
        make_identity(nc, ident_f32)
        nc.vector.tensor_copy(ident_bf, ident_f32)

        cc_in = dram.tile([SQ, DM], BF16)
        cc_out = dram.tile([S, DM], BF16)

        with tc.tile_pool(name=f"qt{it}", bufs=1, side=sd) as qt:
            qkvT_q = qt.tile([P, 4, SQ], F32R)   # Q1,Q1,Q2,Q2, own quarter
            qkvT_k = qt.tile([P, 4, S], F32R)    # K1,K1,K2,K2 full

            # ---------- phase 1: V-chain -> VW quarter -> AllGather --------
            with (
                tc.tile_pool(name=f"vwp{it}", bufs=1, side=sd) as vwp,
                tc.tile_pool(name=f"vstr{it}", bufs=2, side=sd) as vstr,
                tc.tile_pool(name=f"qkvp{it}", bufs=1, side=sd) as qkvp,
            ):
                with (
                    tc.tile_pool(name=f"vps{it}", bufs=3, space="PSUM", side=sd) as vps,
                    tc.tile_pool(name=f"wps{it}", bufs=1, space="PSUM", side=sd) as wps,
                ):
                    xq_sb = vwp.tile([P, KD, SQ], BF16)
                    nc.sync.dma_start(
                        xq_sb, xq_bf.rearrange("(dc p) s -> p dc s", p=P))
                    vw_ps = [wps.tile([P, DM], F32, name=f"vw_ps_{kc}")
                             for kc in range(KQ)]

                    def emit_vt(vb, vc8, wv_sb, n):
                        vt_ps = vps.tile([P, SQ], F32, tag="vtps",
                                         name=f"vtps_{vb}_{vc8}")
                        for dc in range(KD):
                            nc.tensor.matmul(
                                vt_ps, wv_sb[:, dc, ts(vc8, P)], xq_sb[:, dc],
                                start=(dc == 0), stop=(dc == KD - 1))
                        vt_sb = vstr.tile([P, SQ], BF16, tag="vt",
                                          name=f"vt_{vb}_{vc8}")
                        if n % 2 == 0:
                            nc.scalar.activation(vt_sb, vt_ps, IDENT)
                        else:
                            nc.vector.tensor_copy(vt_sb, vt_ps)
                        return vt_sb

                    def emit_vw(vb, vc8, vt_sb, wo_sb):
                        first = vb == 0 and vc8 == 0
                        last = vb == VB - 1 and vc8 == VC8 - 1
                        for kc in range(KQ):
                            nc.tensor.matmul(
                                vw_ps[kc], vt_sb[:, ts(kc, P)],
                                wo_sb[:, vc8, :], start=first, stop=last)

                    pend = []
                    n = 0
                    for vb in range(VB):
                        wv_sb = vstr.tile([P, KD, 1024], BF16, tag="wv",
                                          bufs=2, name=f"wv_{vb}")
                        if vb == 0:
                            for half in range(2):
                                nc.sync.dma_start(
                                    wv_sb[:, :, ts(half, 512)],
                                    wv[:, ds(half * 512, 512)]
                                    .rearrange("(dc p) v -> p dc v", p=P))
                        else:
                            nc.sync.dma_start(
                                wv_sb,
                                wv[:, ts(vb, 1024)]
                                .rearrange("(dc p) v -> p dc v", p=P))
                        wo_sb = vstr.tile([P, VC8, DM], BF16, tag="wo",
                                          bufs=2, name=f"wo_{vb}")
                        nc.sync.dma_start(
                            wo_sb, wo[:, ds(vb * VC8 * DM, VC8 * DM)])
                        for vc8 in range(VC8):
                            vt_sb = emit_vt(vb, vc8, wv_sb, n)
                            n += 1
                            pend.append((vb, vc8, vt_sb, wo_sb))
                            if len(pend) > 1:
                                emit_vw(*pend.pop(0))
                    emit_vw(*pend.pop(0))

                    # evict VW quarter -> cc_in, AllGather in the 4-core group
                    vw_stage = vwp.tile([P, KQ, DM], BF16)
                    for kc in range(KQ):
                        nc.vector.tensor_copy(vw_stage[:, kc], vw_ps[kc])
                    nc.scalar.dma_start(
                        cc_in.rearrange("(kc p) m -> p kc m", p=P), vw_stage)
                    nc.gpsimd.collective_compute(
                        "AllGather", mybir.AluOpType.bypass,
                        ins=[cc_in[:]], outs=[cc_out[:]],
                        replica_groups=[[0, 1, 2, 3], [4, 5, 6, 7]],
                    )
                    # AG-gated gather-in load on the (otherwise idle) gpsimd
                    # queue: its semaphore wait can stall nothing else there
                    nc.gpsimd.dma_start(
                        vw_sb, cc_out.rearrange("(kc p) m -> p kc m", p=P))

                    # qkv-phase loads, streamed during V-chain compute
                    xTs = qkvp.tile([P, KD, S], BF16)
                    wq_sb = qkvp.tile([P, KD, 2 * D], BF16)
                    nc.sync.dma_start(
                        wq_sb, wqkv.rearrange("(dc p) m -> p dc m", p=P))
                    for dc in range(KD):
                        nc.sync.dma_start(xTs[:, dc], xT[ds(dc * P, P), :])

                    # ---- pipeline point: previous body's tail goes here ----
                    yield

                # ---------- phase 2: qkv projection (AllGather shadow) -----
                with tc.tile_pool(name=f"qps{it}", bufs=4,
                                  space="PSUM", side=sd) as qps:
                    for m in range(4):
                        pt = qps.tile([P, SQ], F32, tag="ps")
                        for dc in range(KD):
                            nc.tensor.matmul(
                                pt, wq_sb[:, dc, ts(m, P)], xq_sb[:, dc],
                                start=(dc == 0), stop=(dc == KD - 1))
                        nc.scalar.activation(qkvT_q[:, m], pt, IDENT,
                                             bias=bq_sb[:, m : m + 1])
                    for sn in range(SN):
                        for m in range(4, MQ):
                            pt = qps.tile([P, 512], F32, tag="ps")
                            for dc in range(KD):
                                nc.tensor.matmul(
                                    pt, wq_sb[:, dc, ts(m, P)],
                                    xTs[:, dc, ts(sn, 512)],
                                    start=(dc == 0), stop=(dc == KD - 1))
                            if (sn * 4 + m) % 2 == 0:
                                nc.scalar.activation(
                                    qkvT_k[:, m - 4, ts(sn, 512)], pt, IDENT,
                                    bias=bq_sb[:, m : m + 1])
                            else:
                                nc.vector.tensor_scalar_add(
                                    qkvT_k[:, m - 4, ts(sn, 512)], pt,
                                    bq_sb[:, m : m + 1])

            # ---------- phase 3: scores/softmax/combine/transpose ----------
            with (
                tc.tile_pool(name=f"e1p{it}", bufs=2, side=sd) as e1p,
                tc.tile_pool(name=f"e2p{it}", bufs=2, side=sd) as e2p,
                tc.tile_pool(name=f"pbp{it}", bufs=2, side=sd) as pbp,
                tc.tile_pool(name=f"smallp{it}", bufs=3, side=sd) as smallp,
                tc.tile_pool(name=f"sps{it}", bufs=2, space="PSUM", side=sd) as sps,
            ):
                pend2 = []

                def emit_scores(qb):
                    ets = []
                    sums = []
                    for mi in range(2):
                        pool = e1p if mi == 0 else e2p
                        et = pool.tile([P, S], BF16, tag=f"e{mi}",
                                       name=f"e{mi}_{qb}")
                        st = smallp.tile([P, 2], F32, tag=f"sum{mi}",
                                         name=f"sum{mi}_{qb}")
                        for half in range(2):
                            pt = sps.tile([P, 2, 512], F32, tag="ps",
                                          name=f"ps_{qb}_{mi}_{half}")
                            for knj in range(2):
                                kn = half * 2 + knj
                                for dc in range(2):
                                    nc.tensor.matmul(
                                        pt[:, knj],
                                        qkvT_q[:, 2 * mi + dc, ts(qb, P)],
                                        qkvT_k[:, 2 * mi + dc, ts(kn, 512)],
                                        start=(dc == 0), stop=(dc == 1))
                            nc.scalar.activation(
                                et[:, ts(half, 1024)],
                                pt.rearrange("p a b -> p (a b)"), EXP,
                                scale=SCALE,
                                accum_out=st[:, half : half + 1])
                        ets.append(et)
                        sums.append(st)
                    s1 = smallp.tile([P, 1], F32, tag="s1", name=f"s1_{qb}")
                    nc.vector.reduce_sum(s1, sums[0], axis=AXX)
                    nc.vector.reciprocal(r1s[qb], s1)
                    s2 = smallp.tile([P, 1], F32, tag="s2", name=f"s2_{qb}")
                    nc.vector.reduce_sum(s2, sums[1], axis=AXX)
                    r2 = smallp.tile([P, 1], F32, tag="r2", name=f"r2_{qb}")
                    nc.vector.reciprocal(r2, s2)
                    u = smallp.tile([P, 1], F32, tag="u", name=f"u_{qb}")
                    nc.vector.tensor_mul(u, s1, lam_sb)     # u = -lam*s1
                    r2q = smallp.tile([P, 1], F32, tag="r2q", name=f"r2q_{qb}")
                    nc.vector.tensor_mul(r2q, u, r2)        # r2q = -lam*s1/s2
                    pend2.append((qb, ets, r2q))

                def emit_combine():
                    qb, ets, r2q = pend2.pop(0)
                    pb = pbp.tile([P, S], BF16, tag="pb", name=f"pb_{qb}")
                    nc.vector.affine_then_add(pb, ets[1], ets[0], r2q, 0.0)
                    for kc4 in range(NKC // 4):
                        tp = sps.tile([P, 4, P], BF16, tag="tp",
                                      name=f"tp_{qb}_{kc4}")
                        for j in range(4):
                            kc = kc4 * 4 + j
                            nc.tensor.matmul(tp[:, j], pb[:, ts(kc, P)],
                                             ident_bf, is_transpose=True)
                        nc.vector.tensor_copy(
                            ptile[:, ts(kc4, 4), ts(qb, P)], tp)

                for qb in range(QB):
                    emit_scores(qb)
                    if qb > 0:
                        emit_combine()
                emit_combine()

        # ---- pipeline point: qkvT freed; tail emitted on next resume ------
        yield

        # ---------- tail: p @ VW, final evict (AllGather-gated) ------------
        with (
            tc.tile_pool(name=f"fps{it}", bufs=1, space="PSUM", side=sd) as fps,
            tc.tile_pool(name=f"ofp{it}", bufs=2, side=sd) as ofp,
        ):
            for qb in range(QB):
                ft = fps.tile([P, DM], F32, tag="f", name=f"f_{qb}")
                for kc in range(NKC):
                    nc.tensor.matmul(
                        ft, ptile[:, kc, ts(qb, P)], vw_sb[:, kc, :],
                        start=(kc == 0), stop=(kc == NKC - 1))
                ofsb = ofp.tile([P, DM], F32, tag="of", name=f"of_{qb}")
                nc.scalar.activation(ofsb, ft, IDENT, scale=r1s[qb])
                nc.scalar.dma_start(out[ds(qb * P, P), :], ofsb)


def build_module(n_iters=1):
    nc = bacc.Bacc("TRN2", target_bir_lowering=False, debug=False)
    xT = nc.dram_tensor("xT", (D, S), BF16, kind="ExternalInput").ap()
    xq_bf = nc.dram_tensor("xq_bf", (D, SQ), BF16, kind="ExternalInput").ap()
    wqkv = nc.dram_tensor("wqkv", (D, 2 * D), BF16, kind="ExternalInput").ap()
    wv = nc.dram_tensor("wv", (D, VDIM), BF16, kind="ExternalInput").ap()
    wo = nc.dram_tensor("wo", (P, (VDIM // P) * DM), BF16,
                        kind="ExternalInput").ap()
    lamn = nc.dram_tensor("lamn", (P, 1), F32, kind="ExternalInput").ap()
    bq = nc.dram_tensor("bq", (P, MQ), F32, kind="ExternalInput").ap()
    out = nc.dram_tensor("out", (SQ, DM), F32, kind="ExternalOutput").ap()
    with tile.TileContext(nc) as tc:
        prev = None
        for it in range(n_iters):
            g = kernel_body(tc, it, xT, xq_bf, wqkv, wv, wo, lamn, bq, out)
            next(g)               # phase 1 of body `it`
            if prev is not None:  # tail of body `it-1` lands here
                try:
                    next(prev)
                except StopIteration:
                    pass
            for _ in g:           # phases 2-3, stop at the pre-tail yield
                break
            prev = g
        try:
            next(prev)            # final body's tail
        except StopIteration:
            pass
    nc.compile()
    return nc


_NC = None


def _get_module():
    global _NC
    if _NC is None:
        _NC = build_module()
    return _NC


def host_prep(**inputs):
    """Host-side input prep: returns (in_maps, lam, host_bias)."""
    x = np.asarray(inputs["x"], np.float32)
    Wqkv = np.asarray(inputs["Wqkv"], np.float32)
    bqkv = np.asarray(inputs["bqkv"], np.float32)
    Wv = np.asarray(inputs["Wv"], np.float32)
    bv = np.asarray(inputs["bv"], np.float32)
    Wo = np.asarray(inputs["Wo"], np.float32)
    bo = np.asarray(inputs["bo"], np.float32)
    lq1 = np.asarray(inputs["lq1"], np.float32)
    lk1 = np.asarray(inputs["lk1"], np.float32)
    lq2 = np.asarray(inputs["lq2"], np.float32)
    lk2 = np.asarray(inputs["lk2"], np.float32)

    lam = float(
        np.exp(np.sum(lq1 * lk1, dtype=np.float32))
        - np.exp(np.sum(lq2 * lk2, dtype=np.float32))
        + (LAMBDA_INIT - 0.6 * math.exp(-0.3 * LAYER_INDEX))
    )
    bq_host = np.ascontiguousarray(bqkv.reshape(MQ, P).T)
    lam_host = np.full((P, 1), -lam, np.float32)

    wq_bf = Wqkv.astype(ml_dtypes.bfloat16)
    wv_bf = Wv.astype(ml_dtypes.bfloat16)
    # Wo [8192, 512] -> [128, (vc m)] with vc-major per partition
    wo_bf = np.ascontiguousarray(
        Wo.reshape(VDIM // P, P, DM).transpose(1, 0, 2).reshape(P, -1)
    ).astype(ml_dtypes.bfloat16)

    in_maps = []
    for c in range(8):
        b, g = divmod(c, G)
        xTb = np.ascontiguousarray(x[b].T).astype(ml_dtypes.bfloat16)
        in_maps.append({
            "xT": xTb,
            "xq_bf": np.ascontiguousarray(xTb[:, g * SQ : (g + 1) * SQ]),
            "wqkv": wq_bf,
            "wv": wv_bf,
            "wo": wo_bf,
            "lamn": lam_host,
            "bq": bq_host,
        })
    # sum_k diff_attn[q, :] == 1 - lam exactly, so bv and bo fold into a
    # constant per-output-column correction.
    host_bias = ((1.0 - lam) * bv) @ Wo + bo
    return in_maps, lam, host_bias.astype(np.float32)


def kernel(**inputs):
    in_maps, _lam, host_bias = host_prep(**inputs)
    nc = _get_module()
    res = None
    for attempt in range(3):
        try:
            res = bass_utils.run_bass_kernel_spmd(
                nc, in_maps, core_ids=list(range(8)))
            break
        except Exception:
            # transient NRT_EXEC_UNIT_UNRECOVERABLE flakes have been seen on
            # the first execution of a freshly compiled NEFF; retry
            if attempt == 2:
                raise
            import time
            time.sleep(2.0)
    out = np.empty((B, S, DM), np.float32)
    for c in range(8):
        b, g = divmod(c, G)
        out[b, g * SQ : (g + 1) * SQ, :] = res.results[c]["out"]
    out += host_bias
    return out
